# revision 12
# baseline (speedup 1.0000x reference)
"""Trainium2 Bass kernel for BlockDiagMNIST MLP.

Reference computation (all fp32):
    h  = relu(x @ W1.T + b1)          x:[B,784], W1:[4096,784]    -> [B,4096]
    yb = blockdiag(h, Wb)             Wb:[128,32,32] (h2[b, 32n+o] = sum_k h[b,32n+k] Wb[n,o,k])
    h2 = relu(yb + bb)
    out = h2 @ W3.T + b3              W3:[10,4096]                -> [B,10]

Strategy: pure data-parallel over batch (B=32768 -> 4096 rows/core on 8 cores),
weights replicated.  All matmuls in bf16 (fp32 PSUM accumulation, fp32 biases).
On-chip layout is transposed ("hidden on partitions"): we compute
hT = W1 @ x.T per 512-column batch window.

PE-array packing (v2): layers 2 and 3 underutilize the 128x128 array
(32x32 diagonal blocks / M=10 output), so they are packed as concurrent
tile_position matmuls sharing one streaming slot:
  - L2: two m-tiles (8 32x32 blocks) per slot on a (row,col)-disjoint grid;
    the odd tile's outputs land column-rotated in a second PSUM bank, and the
    rotation is absorbed into the host-side packing of bb and W3.
  - L3: four K=128 chunks per slot at col positions (0,32t); each 10-row
    strip accumulates a quarter of the hidden sum, and the four partial
    strips are summed (+b3) on the host.
This cuts per-window PE slots from 264 to 224.

Host-side prep (free - not on the device timeline): transpose + bf16-cast of
x and weights, packing Wb into 32x32 lhsT strips.
"""

import numpy as np
import ml_dtypes

B = 32768
IN_DIM = 784
HIDDEN = 4096
BLOCK = 32
NUM_BLOCKS = 128
OUT_DIM = 10
NCORES = 8
BC = B // NCORES          # batch rows per core (4096)
WN = 512                  # batch-window columns (one matmul free-dim)
K1 = 6                    # full 128-row K-chunks for layer 1 (features 0..767)
KL = 16                   # leftover K rows (features 768..783), row-group packed
NM = HIDDEN // 128        # 32 hidden tiles (also layer-3 K-chunks)

BF16 = ml_dtypes.bfloat16

_PROGRAM_CACHE = {}


def _perm(m):
    """Partition permutation of hidden rows within m-tile m as laid out on chip.

    Even m-tiles are natural; odd m-tiles are emitted by L2 with their four
    32-row blocks rotated two col-groups to keep the 8 concurrent tile
    positions disjoint: psum partition p = 32c+i holds hidden row
    128m + 32((c+2)%4) + i.
    """
    p = np.arange(128)
    if m % 2 == 0:
        return p
    return 32 * ((p // 32 + 2) % 4) + p % 32


def _build_program(bc=BC):
    """Build (and bacc-compile) the per-core Bass program. bc = batch cols/core."""
    import concourse.mybir as mybir
    import concourse.tile as tile
    from concourse import bacc

    nw = bc // WN
    f32, bf16 = mybir.dt.float32, mybir.dt.bfloat16

    nc = bacc.Bacc("TRN2", target_bir_lowering=False, debug=False)

    xT = nc.dram_tensor("xT", [K1 * 128, bc], bf16, kind="ExternalInput").ap()
    xL = nc.dram_tensor("xL", [128, bc], bf16, kind="ExternalInput").ap()
    w1t = nc.dram_tensor("W1T", [K1 * 128, HIDDEN], bf16, kind="ExternalInput").ap()
    w1l = nc.dram_tensor("W1L", [KL, HIDDEN], bf16, kind="ExternalInput").ap()
    wbig = nc.dram_tensor("Wbig", [128, NM * 128], bf16, kind="ExternalInput").ap()
    w3t = nc.dram_tensor("W3T", [128, NM * OUT_DIM], bf16, kind="ExternalInput").ap()
    # biases packed into one tensor: cols 0..NM-1 = b1, NM..2NM-1 = bb (permuted)
    bcat = nc.dram_tensor("bcat", [128, 2 * NM], f32, kind="ExternalInput").ap()
    # raw PSUM image per window; rows 32t..32t+10 hold partial strip t
    # (summed host-side), other rows are garbage.
    outT = nc.dram_tensor("outT", [128, bc], f32, kind="ExternalOutput").ap()

    Relu = mybir.ActivationFunctionType.Relu
    Add = mybir.AluOpType.add
    Max = mybir.AluOpType.max

    MB = 4          # W1T column-block = MB m-tiles (DMA granularity for overlap)
    NJ = NM // MB   # 8 column blocks

    with tile.TileContext(nc) as tc:
        with (
            tc.tile_pool(name="const", bufs=1) as cpool,
            tc.tile_pool(name="xin", bufs=3) as xpool,
            tc.tile_pool(name="hbuf", bufs=10) as hpool,
            tc.tile_pool(name="h2buf", bufs=10) as h2pool,
            tc.tile_pool(name="obuf", bufs=2) as opool,
            tc.tile_pool(name="ps1", bufs=5, space="PSUM") as ps1,
            tc.tile_pool(name="ps2", bufs=2, space="PSUM") as ps2,
            tc.tile_pool(name="ps3", bufs=1, space="PSUM") as ps3,
        ):
            # HAM warmup on a memset tile: no DMA dependency, so the PE
            # starts as soon as its program is loaded (~6us) instead of
            # waiting for the first input DMA.  bf16 N=512 matmuls bridge
            # until the x/W1 DMAs land; sized so the PE never idles >3.4us
            # (which would re-throttle the clock gate).
            wz = cpool.tile([128, 512], bf16, name="warmzero")
            nc.vector.memset(wz[:], 0.0)
            pw = ps2.tile([128, 512], f32, tag="p2", name="pwarm")
            for _ in range(6):
                nc.tensor.matmul(
                    pw[:], wz[:, 0:128], wz[:], start=True, stop=True,
                )

            xT_r = xT.rearrange("(k p) b -> p k b", p=128)
            w1t_r = w1t.rearrange("(k p) h -> p k h", p=128)

            KH = K1 // 2

            def load_xt(w):
                """Per-window x tiles: two k-half DMAs + the leftover rows."""
                parts = []
                for ci, (k0, k1e) in enumerate(((0, KH), (KH, K1))):
                    t = xpool.tile([128, k1e - k0, WN], bf16, tag=f"xh{ci}",
                                   name=f"xt_{w}_{ci}")
                    nc.sync.dma_start(t[:], xT_r[:, k0:k1e, w * WN:(w + 1) * WN])
                    parts.append((t, k0, k1e))
                tl = xpool.tile([128, WN], bf16, tag="xl", name=f"xl_{w}")
                if w > 0:
                    nc.sync.dma_start(tl[:], xL[:, w * WN:(w + 1) * WN])
                return parts, tl

            # DMA issue order tracks consumption order: bias pack (tiny,
            # primes the queues), then window-0 x and block-0 weights in
            # fine-grained interleaved 2-chunk pieces so the first L1
            # matmul's dependencies land as early as possible, leftovers,
            # then the remaining W1 blocks interleaved with wbig/w3t.
            bc_sb = cpool.tile([128, 2 * NM], f32)
            nc.sync.dma_start(bc_sb[:], bcat)
            b1_sb = bc_sb[:, 0:NM]
            bb_sb = bc_sb[:, NM:2 * NM]

            w1t_t = [None] * NJ

            def load_w1_block(j):
                parts = []
                for ci, (k0, k1e) in enumerate(((0, KH), (KH, K1))):
                    t = cpool.tile([128, k1e - k0, MB * 128], bf16,
                                   name=f"w1t_{j}_{ci}")
                    nc.sync.dma_start(
                        t[:], w1t_r[:, k0:k1e, j * MB * 128:(j + 1) * MB * 128]
                    )
                    parts.append((t, k0, k1e))
                w1t_t[j] = parts

            # window-0 x and W1 block 0, interleaved k-pieces
            x0_parts = []
            w1b0_parts = []
            for k0, k1e in ((0, 2), (2, 4), (4, K1)):
                tx = cpool.tile([128, k1e - k0, WN], bf16, name=f"x0_{k0}")
                nc.sync.dma_start(tx[:], xT_r[:, k0:k1e, 0:WN])
                x0_parts.append((tx, k0, k1e))
                tw = cpool.tile([128, k1e - k0, MB * 128], bf16, name=f"w1b0_{k0}")
                nc.sync.dma_start(tw[:], w1t_r[:, k0:k1e, 0:MB * 128])
                w1b0_parts.append((tw, k0, k1e))
            w1t_t[0] = w1b0_parts
            xl0 = xpool.tile([128, WN], bf16, tag="xl", name="xl_0")
            xts = {0: (x0_parts, xl0)}

            # window-0 leftover x rows + compact leftover weights (the 16
            # leftover feature rows, replicated into the 4 PE row groups by
            # 4 small DMAs instead of shipping a 128-row mostly-zero tensor)
            nc.sync.dma_start(xl0[:], xL[:, 0:WN])
            w1l_sb = cpool.tile([128, HIDDEN], bf16)
            for j in range(4):
                nc.sync.dma_start(w1l_sb[32 * j:32 * j + KL, :], w1l)

            wbig_sb = cpool.tile([128, NM * 128], bf16)
            w3t_sb = cpool.tile([128, NM * OUT_DIM], bf16)
            load_w1_block(1)
            nc.sync.dma_start(wbig_sb[:], wbig)
            load_w1_block(2)
            nc.sync.dma_start(w3t_sb[:], w3t)
            for j in range(3, NJ):
                load_w1_block(j)

            # Software pipeline, emitted per group G of 4 m-tiles:
            #   L1(G, tiles 0-1) | L2 pair | L1(G, tiles 2-3) | L2 pair |
            #   L1 leftovers | relu evac | L3 slot(G-2)
            # The two L2 pair-slots of group G-1 are spaced ~2.6us apart so
            # the two single-buffered ps2 banks are always evacuated in time.
            NGW = NM // 4
            NGTOT = nw * NGW
            pos = {}   # window -> psum accumulator for layer 3
            hs = {}    # global M -> h tile (layer-1 output)
            h2s = {}   # global M -> h2 tile (layer-2 output)

            def _pick(parts, k):
                for t, k0, k1e in parts:
                    if k0 <= k < k1e:
                        return t, k - k0
                raise KeyError(k)

            def emit_l1_tiles(G, js, p1s):
                """Full-K matmuls for m-tiles {4g+j : j in js} of group G."""
                w, g = divmod(G, NGW)
                xparts, _ = xts[w]
                for j in js:
                    m = 4 * g + j
                    p1 = ps1.tile([128, WN], f32, tag="p1", name=f"p1_{G}_{j}")
                    for k in range(K1):
                        wt, wk = _pick(w1t_t[m // MB], k)
                        xt_c, xk = _pick(xparts, k)
                        nc.tensor.matmul(
                            p1[:],
                            wt[:, wk, (m % MB) * 128:(m % MB + 1) * 128],
                            xt_c[:, xk, :],
                            start=(k == 0),
                            stop=False,
                        )
                    p1s[j] = p1

            def emit_l1_tail(G, p1s):
                """Leftover K=16 matmuls (4 concurrent row-groups) + relu."""
                w, g = divmod(G, NGW)
                _, xl = xts[w]
                for j in range(4):
                    m = 4 * g + j
                    nc.tensor.matmul(
                        p1s[j][:],
                        w1l_sb[32 * j:32 * j + KL, m * 128:(m + 1) * 128],
                        xl[32 * j:32 * j + KL, :],
                        start=False,
                        stop=True,
                        tile_position=(32 * j, 0),
                    )
                for j in range(4):
                    m = 4 * g + j
                    h = hpool.tile([128, WN], bf16, tag="h", name=f"h_{G}_{j}")
                    nc.scalar.activation(h[:], p1s[j][:], Relu, bias=b1_sb[:, m:m + 1])
                    hs[w * NM + m] = h

            def emit_l2(M):
                """Dense block-diagonal L2 matmul for one m-tile: a full
                128x128 lhsT packs the tile's four 32x32 diagonal blocks, so
                the weight load rides the background buffer stall-free."""
                w, m = divmod(M, NM)
                p2 = ps2.tile([128, WN], f32, tag="p2", name=f"p2_{M}")
                nc.tensor.matmul(
                    p2[:],
                    wbig_sb[:, m * 128:(m + 1) * 128],
                    hs.pop(M)[:],
                    start=True,
                    stop=True,
                )
                h2 = h2pool.tile([128, WN], bf16, tag="h2", name=f"h2_{M}")
                nc.vector.tensor_scalar(h2[:], p2[:], bb_sb[:, m:m + 1], 0.0, Add, Max)
                h2s[M] = h2

            def emit_l3_slot(S):
                """One slot: 4 concurrent M=10 col-tiles; strip t accumulates
                m-tiles == t (mod 4) over the window's 8 slots."""
                w, s = divmod(S, NGW)
                if s == 0:
                    pos[w] = ps3.tile([128, WN], f32, tag="po", name=f"po_{w}")
                for t in range(4):
                    m = 4 * s + t
                    nc.tensor.matmul(
                        pos[w][32 * t:32 * t + OUT_DIM, :],
                        w3t_sb[:, OUT_DIM * m:OUT_DIM * (m + 1)],
                        h2s.pop(w * NM + m)[:],
                        start=(s == 0),
                        stop=(s == NGW - 1),
                        tile_position=(0, 32 * t),
                        skip_group_check=True,
                    )
                if s == NGW - 1:
                    ot = opool.tile([128, WN], f32, tag="ot", name=f"ot_{w}")
                    nc.vector.tensor_scalar_add(ot[:], pos.pop(w)[:], 0.0)
                    nc.sync.dma_start(outT[:, w * WN:(w + 1) * WN], ot[:])

            # Dense L2 matmuls blend stall-free into the L1 stream; the
            # leftover and packed-L3 slots are the only tile_position slots
            # and sit clustered at the group boundary.
            for G in range(NGTOT + 2):
                p1s = {}
                if G < NGTOT:
                    w, g = divmod(G, NGW)
                    if g == 0 and w not in xts:
                        xts[w] = load_xt(w)
                    emit_l1_tiles(G, (0, 1), p1s)
                if 1 <= G <= NGTOT:
                    for j in range(2):
                        emit_l2(4 * (G - 1) + j)
                if G < NGTOT:
                    emit_l1_tiles(G, (2, 3), p1s)
                if 1 <= G <= NGTOT:
                    for j in range(2, 4):
                        emit_l2(4 * (G - 1) + j)
                if G < NGTOT:
                    emit_l1_tail(G, p1s)
                if G >= 2:
                    emit_l3_slot(G - 2)

    nc.compile()
    return nc


def _get_program(bc=BC):
    if bc not in _PROGRAM_CACHE:
        _PROGRAM_CACHE[bc] = _build_program(bc)
    return _PROGRAM_CACHE[bc]


def _prep_weights(W1, b1, Wb, bb, W3, b3):
    """Host-side packing of replicated weights into device layouts."""
    W1 = np.asarray(W1, dtype=np.float32)
    Wb = np.asarray(Wb, dtype=np.float32)
    W3 = np.asarray(W3, dtype=np.float32)

    # W1T [768, 4096] = first 768 input features; W1L [16, 4096] = the
    # leftover feature rows (replicated into 4 PE row groups device-side).
    W1T = np.ascontiguousarray(W1.T[:K1 * 128]).astype(BF16)
    W1L = np.ascontiguousarray(W1.T[K1 * 128:IN_DIM]).astype(BF16)

    # Wbig [128, 32*128]: col group m holds blockdiag(Wb[4m+j].T), j=0..3.
    Wbig = np.zeros((128, NM * 128), dtype=BF16)
    for m in range(NM):
        for j in range(4):
            Wbig[32 * j:32 * j + 32, 128 * m + 32 * j:128 * m + 32 * j + 32] = \
                Wb[4 * m + j].T.astype(BF16)

    # W3T [128, NM*10]: per m-tile a [128, 10] lhsT with the m-tile's hidden
    # rows in on-chip (possibly rotated) partition order.
    W3T = np.zeros((128, NM * OUT_DIM), dtype=BF16)
    bb = np.asarray(bb, np.float32)
    bb2 = np.zeros((128, NM), dtype=np.float32)
    for m in range(NM):
        pm = 128 * m + np.arange(128)
        W3T[:, OUT_DIM * m:OUT_DIM * (m + 1)] = W3[:, pm].T.astype(BF16)
        bb2[:, m] = bb[pm]

    bcat = np.zeros((128, 2 * NM), dtype=np.float32)
    bcat[:, 0:NM] = np.asarray(b1, np.float32).reshape(NM, 128).T
    bcat[:, NM:2 * NM] = bb2
    return dict(W1T=W1T, W1L=W1L, Wbig=Wbig, W3T=W3T, bcat=bcat)


def _prep_x_shard(x, c, ncores=NCORES, bc=BC):
    xs = np.asarray(x[c * bc:(c + 1) * bc], dtype=np.float32).T.astype(BF16)  # [784, bc]
    xT = np.ascontiguousarray(xs[:K1 * 128])
    xLs = np.zeros((128, bc), dtype=BF16)
    for j in range(4):
        xLs[32 * j:32 * j + KL] = xs[K1 * 128:IN_DIM]
    return xT, xLs


def run(x, W1, b1, Wb, bb, W3, b3, trace=False, tmpdir=None):
    """Run on 8 cores; returns (out [B,10] fp32, BassKernelResults)."""
    from concourse.bass_utils import run_bass_kernel_spmd

    nc = _get_program()
    wmap = _prep_weights(W1, b1, Wb, bb, W3, b3)
    in_maps = []
    for c in range(NCORES):
        m = dict(wmap)
        m["xT"], m["xL"] = _prep_x_shard(np.asarray(x), c)
        in_maps.append(m)

    res = run_bass_kernel_spmd(
        nc, in_maps, core_ids=list(range(NCORES)), trace=trace, tmpdir=tmpdir
    )
    b3f = np.asarray(b3, np.float32)
    outs = []
    for r in res.results:
        raw = np.asarray(r["outT"])  # [128, BC]; strips at rows 32t..32t+10
        acc = sum(raw[32 * t:32 * t + OUT_DIM] for t in range(4))
        outs.append(acc.T + b3f)
    out = np.concatenate(outs, axis=0).astype(np.float32)
    return out, res


def kernel(x, W1, b1, Wb, bb, W3, b3):
    out, _ = run(x, W1, b1, Wb, bb, W3, b3, trace=False)
    return out


# revision 13
# speedup vs baseline: 1.0553x; 1.0553x over previous
"""Trainium2 Bass kernel for BlockDiagMNIST MLP.

Reference computation (all fp32):
    h  = relu(x @ W1.T + b1)          x:[B,784], W1:[4096,784]    -> [B,4096]
    yb = blockdiag(h, Wb)             Wb:[128,32,32] (h2[b, 32n+o] = sum_k h[b,32n+k] Wb[n,o,k])
    h2 = relu(yb + bb)
    out = h2 @ W3.T + b3              W3:[10,4096]                -> [B,10]

Strategy: pure data-parallel over batch (B=32768 -> 4096 rows/core on 8 cores),
weights replicated.  All matmuls in bf16 (fp32 PSUM accumulation, fp32 biases).
On-chip layout is transposed ("hidden on partitions"): we compute
hT = W1 @ x.T per 512-column batch window.

PE-array packing: layers 2 and 3 underutilize the 128x128 array (32x32
diagonal blocks / M=10 output), so they are packed as concurrent
tile_position matmuls sharing one streaming slot:
  - L2: two m-tiles (8 32x32 blocks) per slot on a (row,col)-disjoint grid;
    the odd tile's outputs land column-rotated in a second PSUM bank, and the
    rotation is absorbed into the host-side packing of bb and W3.
  - The leftover K=16 matmuls ride in the same pair slots as [16,32]
    col-strip pieces at the 8 unused (row,col) positions, closing each
    L1 accumulation without a dedicated slot.
  - L3: four K=128 chunks per slot at col positions (0,32t); each 10-row
    strip accumulates a quarter of the hidden sum, and the four partial
    strips are summed (+b3) on the host.

Host-side prep (free - not on the device timeline): transpose + bf16-cast of
x and weights, packing Wb into 32x32 lhsT strips.
"""

import numpy as np
import ml_dtypes

B = 32768
IN_DIM = 784
HIDDEN = 4096
BLOCK = 32
NUM_BLOCKS = 128
OUT_DIM = 10
NCORES = 8
BC = B // NCORES          # batch rows per core (4096)
WN = 512                  # batch-window columns (one matmul free-dim)
K1 = 6                    # full 128-row K-chunks for layer 1 (features 0..767)
KL = 16                   # leftover K rows (features 768..783), row-group packed
NM = HIDDEN // 128        # 32 hidden tiles (also layer-3 K-chunks)

BF16 = ml_dtypes.bfloat16

_PROGRAM_CACHE = {}


def _perm(m):
    """Partition permutation of hidden rows within m-tile m as laid out on chip.

    Even m-tiles are natural; odd m-tiles are emitted by L2 with their four
    32-row blocks rotated two col-groups to keep the 8 concurrent tile
    positions disjoint: psum partition p = 32c+i holds hidden row
    128m + 32((c+2)%4) + i.
    """
    p = np.arange(128)
    if m % 2 == 0:
        return p
    return 32 * ((p // 32 + 2) % 4) + p % 32


def _build_program(bc=BC):
    """Build (and bacc-compile) the per-core Bass program. bc = batch cols/core."""
    import concourse.mybir as mybir
    import concourse.tile as tile
    from concourse import bacc

    nw = bc // WN
    f32, bf16 = mybir.dt.float32, mybir.dt.bfloat16

    nc = bacc.Bacc("TRN2", target_bir_lowering=False, debug=False)

    xT = nc.dram_tensor("xT", [K1 * 128, bc], bf16, kind="ExternalInput").ap()
    xL = nc.dram_tensor("xL", [128, bc], bf16, kind="ExternalInput").ap()
    w1t = nc.dram_tensor("W1T", [K1 * 128, HIDDEN], bf16, kind="ExternalInput").ap()
    w1l = nc.dram_tensor("W1L", [KL, HIDDEN], bf16, kind="ExternalInput").ap()
    wbig = nc.dram_tensor("Wbig", [128, NM * 32], bf16, kind="ExternalInput").ap()
    w3t = nc.dram_tensor("W3T", [128, NM * OUT_DIM], bf16, kind="ExternalInput").ap()
    # biases packed into one tensor: cols 0..NM-1 = b1, NM..2NM-1 = bb (permuted)
    bcat = nc.dram_tensor("bcat", [128, 2 * NM], f32, kind="ExternalInput").ap()
    # raw PSUM image per window; rows 32t..32t+10 hold partial strip t
    # (summed host-side), other rows are garbage.
    outT = nc.dram_tensor("outT", [128, bc], f32, kind="ExternalOutput").ap()

    Relu = mybir.ActivationFunctionType.Relu
    Add = mybir.AluOpType.add
    Max = mybir.AluOpType.max

    MB = 4          # W1T column-block = MB m-tiles (DMA granularity for overlap)
    NJ = NM // MB   # 8 column blocks

    with tile.TileContext(nc) as tc:
        with (
            tc.tile_pool(name="const", bufs=1) as cpool,
            tc.tile_pool(name="xin", bufs=3) as xpool,
            tc.tile_pool(name="hbuf", bufs=10) as hpool,
            tc.tile_pool(name="h2buf", bufs=10) as h2pool,
            tc.tile_pool(name="obuf", bufs=2) as opool,
            tc.tile_pool(name="ps1", bufs=5, space="PSUM") as ps1,
            tc.tile_pool(name="ps2", bufs=1, space="PSUM") as ps2,
            tc.tile_pool(name="ps3", bufs=1, space="PSUM") as ps3,
        ):
            # HAM warmup on a memset tile: no DMA dependency, so the PE
            # starts as soon as its program is loaded (~8us); a few bf16
            # N=512 matmuls bridge until the first x/W1 DMA pieces land.
            wz = cpool.tile([128, 512], bf16, name="warmzero")
            nc.vector.memset(wz[:], 0.0)
            pw = ps2.tile([128, 512], f32, tag="p2a", name="pwarm")
            for _ in range(6):
                nc.tensor.matmul(
                    pw[:], wz[:, 0:128], wz[:], start=True, stop=True,
                )

            xT_r = xT.rearrange("(k p) b -> p k b", p=128)
            w1t_r = w1t.rearrange("(k p) h -> p k h", p=128)

            KH = K1 // 2

            def load_xt(w):
                """Per-window x tiles: two k-half DMAs + the leftover rows."""
                parts = []
                for ci, (k0, k1e) in enumerate(((0, KH), (KH, K1))):
                    t = xpool.tile([128, k1e - k0, WN], bf16, tag=f"xh{ci}",
                                   name=f"xt_{w}_{ci}")
                    nc.sync.dma_start(t[:], xT_r[:, k0:k1e, w * WN:(w + 1) * WN])
                    parts.append((t, k0, k1e))
                tl = xpool.tile([128, WN], bf16, tag="xl", name=f"xl_{w}")
                if w > 0:
                    nc.sync.dma_start(tl[:], xL[:, w * WN:(w + 1) * WN])
                return parts, tl

            # DMA issue order tracks consumption order: bias pack (tiny,
            # primes the queues), then window-0 x and block-0 weights in
            # fine-grained interleaved 2-chunk pieces so the first L1
            # matmul's dependencies land as early as possible, leftovers,
            # then the remaining W1 blocks interleaved with wbig/w3t.
            bc_sb = cpool.tile([128, 2 * NM], f32)
            nc.sync.dma_start(bc_sb[:], bcat)
            b1_sb = bc_sb[:, 0:NM]
            bb_sb = bc_sb[:, NM:2 * NM]

            w1t_t = [None] * NJ

            def load_w1_block(j):
                parts = []
                for ci, (k0, k1e) in enumerate(((0, KH), (KH, K1))):
                    t = cpool.tile([128, k1e - k0, MB * 128], bf16,
                                   name=f"w1t_{j}_{ci}")
                    nc.sync.dma_start(
                        t[:], w1t_r[:, k0:k1e, j * MB * 128:(j + 1) * MB * 128]
                    )
                    parts.append((t, k0, k1e))
                w1t_t[j] = parts

            # window-0 x and W1 block 0, interleaved k-pieces
            x0_parts = []
            w1b0_parts = []
            for k0, k1e in ((0, 2), (2, 4), (4, K1)):
                tx = cpool.tile([128, k1e - k0, WN], bf16, name=f"x0_{k0}")
                nc.sync.dma_start(tx[:], xT_r[:, k0:k1e, 0:WN])
                x0_parts.append((tx, k0, k1e))
                tw = cpool.tile([128, k1e - k0, MB * 128], bf16, name=f"w1b0_{k0}")
                nc.sync.dma_start(tw[:], w1t_r[:, k0:k1e, 0:MB * 128])
                w1b0_parts.append((tw, k0, k1e))
            w1t_t[0] = w1b0_parts
            xl0 = xpool.tile([128, WN], bf16, tag="xl", name="xl_0")
            xts = {0: (x0_parts, xl0)}

            # window-0 leftover x rows + compact leftover weights (the 16
            # leftover feature rows, replicated into the 4 PE row groups by
            # 4 small DMAs instead of shipping a 128-row mostly-zero tensor)
            nc.sync.dma_start(xl0[:], xL[:, 0:WN])
            w1l_sb = cpool.tile([128, HIDDEN], bf16)
            for j in range(4):
                nc.sync.dma_start(w1l_sb[32 * j:32 * j + KL, :], w1l)

            wbig_sb = cpool.tile([128, NM * 32], bf16)
            w3t_sb = cpool.tile([128, NM * OUT_DIM], bf16)
            load_w1_block(1)
            nc.sync.dma_start(wbig_sb[:], wbig)
            load_w1_block(2)
            nc.sync.dma_start(w3t_sb[:], w3t)
            for j in range(3, NJ):
                load_w1_block(j)

            # Software pipeline, per group G of 4 m-tiles:
            #   L1(G, tiles 0-1) | pair+lft | L1(G, tiles 2-3) | pair+lft | L3
            # Each pair slot is 16 concurrent tile_position matmuls: 8 L2
            # blocks for two of G-1's m-tiles plus the 8 leftover pieces
            # that close the accumulation of two of G's m-tiles.
            NGW = NM // 4
            NGTOT = nw * NGW
            pos = {}   # window -> psum accumulator for layer 3
            hs = {}    # global M -> h tile (layer-1 output)
            h2s = {}   # global M -> h2 tile (layer-2 output)

            def _pick(parts, k):
                for t, k0, k1e in parts:
                    if k0 <= k < k1e:
                        return t, k - k0
                raise KeyError(k)

            def emit_l1_tiles(G, js, p1s):
                """Full-K matmuls for m-tiles {4g+j : j in js} of group G."""
                w, g = divmod(G, NGW)
                xparts, _ = xts[w]
                for j in js:
                    m = 4 * g + j
                    p1 = ps1.tile([128, WN], f32, tag="p1", name=f"p1_{G}_{j}")
                    for k in range(K1):
                        wt, wk = _pick(w1t_t[m // MB], k)
                        xt_c, xk = _pick(xparts, k)
                        nc.tensor.matmul(
                            p1[:],
                            wt[:, wk, (m % MB) * 128:(m % MB + 1) * 128],
                            xt_c[:, xk, :],
                            start=(k == 0),
                            stop=False,
                        )
                    p1s[j] = p1

            def emit_lft(G, js, p1s):
                """Leftover K=16 pieces closing tiles {4g+j : j in js} of G,
                as [16,32] col-strips at the 8 (row,col) grid cells the L2
                pair slot does not occupy (rows c+1 / c+3 for strip c)."""
                w, g = divmod(G, NGW)
                _, xl = xts[w]
                for i, j in enumerate(js):
                    m = 4 * g + j
                    for c in range(4):
                        r = (c + (1 if i == 0 else 3)) % 4
                        nc.tensor.matmul(
                            p1s[j][32 * c:32 * c + 32, :],
                            w1l_sb[32 * r:32 * r + KL,
                                   m * 128 + 32 * c:m * 128 + 32 * c + 32],
                            xl[32 * r:32 * r + KL, :],
                            start=False,
                            stop=True,
                            tile_position=(32 * r, 32 * c),
                            skip_group_check=True,
                        )

            def emit_act(G, js, p1s):
                w, g = divmod(G, NGW)
                for j in js:
                    m = 4 * g + j
                    h = hpool.tile([128, WN], bf16, tag="h", name=f"h_{G}_{j}")
                    nc.scalar.activation(h[:], p1s[j][:], Relu, bias=b1_sb[:, m:m + 1])
                    hs[w * NM + m] = h

            def emit_l2_pair(P):
                """8 concurrent 32x32 block matmuls for m-tiles (2p, 2p+1);
                even tile on the diagonal into bank a, odd tile col-rotated
                by 2 into bank b."""
                w, p = divmod(P, NM // 2)
                m0, m1 = 2 * p, 2 * p + 1
                p2a = ps2.tile([128, WN], f32, tag="p2a", name=f"p2a_{P}")
                p2b = ps2.tile([128, WN], f32, tag="p2b", name=f"p2b_{P}")
                h0 = hs.pop(w * NM + m0)
                h1 = hs.pop(w * NM + m1)
                for j in range(4):
                    nc.tensor.matmul(
                        p2a[32 * j:32 * j + 32, :],
                        wbig_sb[32 * j:32 * j + 32, 32 * m0:32 * m0 + 32],
                        h0[32 * j:32 * j + 32, :],
                        start=True,
                        stop=True,
                        tile_position=(32 * j, 32 * j),
                    )
                for j in range(4):
                    c = (j + 2) % 4
                    nc.tensor.matmul(
                        p2b[32 * c:32 * c + 32, :],
                        wbig_sb[32 * j:32 * j + 32, 32 * m1:32 * m1 + 32],
                        h1[32 * j:32 * j + 32, :],
                        start=True,
                        stop=True,
                        tile_position=(32 * j, 32 * c),
                    )
                for m, p2 in ((m0, p2a), (m1, p2b)):
                    h2 = h2pool.tile([128, WN], bf16, tag="h2", name=f"h2_{P}_{m}")
                    nc.vector.tensor_scalar(h2[:], p2[:], bb_sb[:, m:m + 1], 0.0, Add, Max)
                    h2s[w * NM + m] = h2

            def emit_l3_slot(S):
                """One slot: 4 concurrent M=10 col-tiles; strip t accumulates
                m-tiles == t (mod 4) over the window's 8 slots."""
                w, s = divmod(S, NGW)
                if s == 0:
                    pos[w] = ps3.tile([128, WN], f32, tag="po", name=f"po_{w}")
                for t in range(4):
                    m = 4 * s + t
                    nc.tensor.matmul(
                        pos[w][32 * t:32 * t + OUT_DIM, :],
                        w3t_sb[:, OUT_DIM * m:OUT_DIM * (m + 1)],
                        h2s.pop(w * NM + m)[:],
                        start=(s == 0),
                        stop=(s == NGW - 1),
                        tile_position=(0, 32 * t),
                        skip_group_check=True,
                    )
                if s == NGW - 1:
                    ot = opool.tile([128, WN], f32, tag="ot", name=f"ot_{w}")
                    nc.vector.tensor_scalar_add(ot[:], pos.pop(w)[:], 0.0)
                    nc.sync.dma_start(outT[:, w * WN:(w + 1) * WN], ot[:])

            for G in range(NGTOT + 2):
                p1s = {}
                if G < NGTOT:
                    w, g = divmod(G, NGW)
                    if g == 0 and w not in xts:
                        xts[w] = load_xt(w)
                    emit_l1_tiles(G, (0, 1), p1s)
                if 1 <= G <= NGTOT:
                    emit_l2_pair(2 * (G - 1))
                if G < NGTOT:
                    emit_lft(G, (0, 1), p1s)
                    emit_act(G, (0, 1), p1s)
                    emit_l1_tiles(G, (2, 3), p1s)
                if 1 <= G <= NGTOT:
                    emit_l2_pair(2 * (G - 1) + 1)
                if G < NGTOT:
                    emit_lft(G, (2, 3), p1s)
                    emit_act(G, (2, 3), p1s)
                if G >= 2:
                    emit_l3_slot(G - 2)

    nc.compile()
    return nc


def _get_program(bc=BC):
    if bc not in _PROGRAM_CACHE:
        _PROGRAM_CACHE[bc] = _build_program(bc)
    return _PROGRAM_CACHE[bc]


def _prep_weights(W1, b1, Wb, bb, W3, b3):
    """Host-side packing of replicated weights into device layouts."""
    W1 = np.asarray(W1, dtype=np.float32)
    Wb = np.asarray(Wb, dtype=np.float32)
    W3 = np.asarray(W3, dtype=np.float32)

    # W1T [768, 4096] = first 768 input features; W1L [16, 4096] = the
    # leftover feature rows (replicated into 4 PE row groups device-side).
    W1T = np.ascontiguousarray(W1.T[:K1 * 128]).astype(BF16)
    W1L = np.ascontiguousarray(W1.T[K1 * 128:IN_DIM]).astype(BF16)

    # Wbig [128, 32*32]: partition strip j, col group m holds Wb[4m+j].T.
    Wbig = np.zeros((128, NM * 32), dtype=BF16)
    for m in range(NM):
        for j in range(4):
            Wbig[32 * j:32 * j + 32, 32 * m:32 * m + 32] = Wb[4 * m + j].T.astype(BF16)

    # W3T [128, NM*10]: per m-tile a [128, 10] lhsT with the m-tile's hidden
    # rows in on-chip (possibly rotated) partition order.
    W3T = np.zeros((128, NM * OUT_DIM), dtype=BF16)
    bb = np.asarray(bb, np.float32)
    bb2 = np.zeros((128, NM), dtype=np.float32)
    for m in range(NM):
        pm = 128 * m + _perm(m)
        W3T[:, OUT_DIM * m:OUT_DIM * (m + 1)] = W3[:, pm].T.astype(BF16)
        bb2[:, m] = bb[pm]

    bcat = np.zeros((128, 2 * NM), dtype=np.float32)
    bcat[:, 0:NM] = np.asarray(b1, np.float32).reshape(NM, 128).T
    bcat[:, NM:2 * NM] = bb2
    return dict(W1T=W1T, W1L=W1L, Wbig=Wbig, W3T=W3T, bcat=bcat)


def _prep_x_shard(x, c, ncores=NCORES, bc=BC):
    xs = np.asarray(x[c * bc:(c + 1) * bc], dtype=np.float32).T.astype(BF16)  # [784, bc]
    xT = np.ascontiguousarray(xs[:K1 * 128])
    xLs = np.zeros((128, bc), dtype=BF16)
    for j in range(4):
        xLs[32 * j:32 * j + KL] = xs[K1 * 128:IN_DIM]
    return xT, xLs


def run(x, W1, b1, Wb, bb, W3, b3, trace=False, tmpdir=None):
    """Run on 8 cores; returns (out [B,10] fp32, BassKernelResults)."""
    from concourse.bass_utils import run_bass_kernel_spmd

    nc = _get_program()
    wmap = _prep_weights(W1, b1, Wb, bb, W3, b3)
    in_maps = []
    for c in range(NCORES):
        m = dict(wmap)
        m["xT"], m["xL"] = _prep_x_shard(np.asarray(x), c)
        in_maps.append(m)

    res = run_bass_kernel_spmd(
        nc, in_maps, core_ids=list(range(NCORES)), trace=trace, tmpdir=tmpdir
    )
    b3f = np.asarray(b3, np.float32)
    outs = []
    for r in res.results:
        raw = np.asarray(r["outT"])  # [128, BC]; strips at rows 32t..32t+10
        acc = sum(raw[32 * t:32 * t + OUT_DIM] for t in range(4))
        outs.append(acc.T + b3f)
    out = np.concatenate(outs, axis=0).astype(np.float32)
    return out, res


def kernel(x, W1, b1, Wb, bb, W3, b3):
    out, _ = run(x, W1, b1, Wb, bb, W3, b3, trace=False)
    return out


# revision 14
# speedup vs baseline: 1.0594x; 1.0039x over previous
"""Trainium2 Bass kernel for BlockDiagMNIST MLP.

Reference computation (all fp32):
    h  = relu(x @ W1.T + b1)          x:[B,784], W1:[4096,784]    -> [B,4096]
    yb = blockdiag(h, Wb)             Wb:[128,32,32] (h2[b, 32n+o] = sum_k h[b,32n+k] Wb[n,o,k])
    h2 = relu(yb + bb)
    out = h2 @ W3.T + b3              W3:[10,4096]                -> [B,10]

Strategy: pure data-parallel over batch (B=32768 -> 4096 rows/core on 8 cores),
weights replicated.  All matmuls in bf16 (fp32 PSUM accumulation, fp32 biases).
On-chip layout is transposed ("hidden on partitions"): we compute
hT = W1 @ x.T per 512-column batch window.

PE-array packing: layers 2 and 3 underutilize the 128x128 array (32x32
diagonal blocks / M=10 output), so they are packed as concurrent
tile_position matmuls sharing one streaming slot:
  - L2: two m-tiles (8 32x32 blocks) per slot on a (row,col)-disjoint grid;
    the odd tile's outputs land column-rotated in a second PSUM bank, and the
    rotation is absorbed into the host-side packing of bb and W3.
  - The leftover K=16 matmuls ride in the same pair slots as [16,32]
    col-strip pieces at the 8 unused (row,col) positions, closing each
    L1 accumulation without a dedicated slot.
  - L3: four K=128 chunks per slot at col positions (0,32t); each 10-row
    strip accumulates a quarter of the hidden sum, and the four partial
    strips are summed (+b3) on the host.

Host-side prep (free - not on the device timeline): transpose + bf16-cast of
x and weights, packing Wb into 32x32 lhsT strips.
"""

import numpy as np
import ml_dtypes

B = 32768
IN_DIM = 784
HIDDEN = 4096
BLOCK = 32
NUM_BLOCKS = 128
OUT_DIM = 10
NCORES = 8
BC = B // NCORES          # batch rows per core (4096)
WN = 512                  # batch-window columns (one matmul free-dim)
K1 = 6                    # full 128-row K-chunks for layer 1 (features 0..767)
KL = 16                   # leftover K rows (features 768..783), row-group packed
NM = HIDDEN // 128        # 32 hidden tiles (also layer-3 K-chunks)

BF16 = ml_dtypes.bfloat16

_PROGRAM_CACHE = {}


def _perm(m):
    """Partition permutation of hidden rows within m-tile m as laid out on chip.

    Even m-tiles are natural; odd m-tiles are emitted by L2 with their four
    32-row blocks rotated two col-groups to keep the 8 concurrent tile
    positions disjoint: psum partition p = 32c+i holds hidden row
    128m + 32((c+2)%4) + i.
    """
    p = np.arange(128)
    if m % 2 == 0:
        return p
    return 32 * ((p // 32 + 2) % 4) + p % 32


def _build_program(bc=BC):
    """Build (and bacc-compile) the per-core Bass program. bc = batch cols/core."""
    import concourse.mybir as mybir
    import concourse.tile as tile
    from concourse import bacc

    nw = bc // WN
    f32, bf16 = mybir.dt.float32, mybir.dt.bfloat16

    nc = bacc.Bacc("TRN2", target_bir_lowering=False, debug=False)

    xT = nc.dram_tensor("xT", [K1 * 128, bc], bf16, kind="ExternalInput").ap()
    xL = nc.dram_tensor("xL", [128, bc], bf16, kind="ExternalInput").ap()
    w1t = nc.dram_tensor("W1T", [K1 * 128, HIDDEN], bf16, kind="ExternalInput").ap()
    w1l = nc.dram_tensor("W1L", [KL, HIDDEN], bf16, kind="ExternalInput").ap()
    wbig = nc.dram_tensor("Wbig", [128, NM * 32], bf16, kind="ExternalInput").ap()
    w3t = nc.dram_tensor("W3T", [128, NM * OUT_DIM], bf16, kind="ExternalInput").ap()
    # biases packed into one tensor: cols 0..NM-1 = b1, NM..2NM-1 = bb (permuted)
    bcat = nc.dram_tensor("bcat", [128, 2 * NM], f32, kind="ExternalInput").ap()
    # raw PSUM image per window; rows 32t..32t+10 hold partial strip t
    # (summed host-side), other rows are garbage.
    outT = nc.dram_tensor("outT", [128, bc], f32, kind="ExternalOutput").ap()

    Relu = mybir.ActivationFunctionType.Relu
    Add = mybir.AluOpType.add
    Max = mybir.AluOpType.max

    MB = 4          # W1T column-block = MB m-tiles (DMA granularity for overlap)
    NJ = NM // MB   # 8 column blocks

    with tile.TileContext(nc) as tc:
        with (
            tc.tile_pool(name="const", bufs=1) as cpool,
            tc.tile_pool(name="xin", bufs=3) as xpool,
            tc.tile_pool(name="hbuf", bufs=10) as hpool,
            tc.tile_pool(name="h2buf", bufs=14) as h2pool,
            tc.tile_pool(name="obuf", bufs=2) as opool,
            tc.tile_pool(name="ps1", bufs=5, space="PSUM") as ps1,
            tc.tile_pool(name="ps2", bufs=1, space="PSUM") as ps2,
            tc.tile_pool(name="ps3", bufs=1, space="PSUM") as ps3,
        ):
            # HAM warmup on a memset tile: no DMA dependency, so the PE
            # starts as soon as its program is loaded (~8us); a few bf16
            # N=512 matmuls bridge until the first x/W1 DMA pieces land.
            wz = cpool.tile([128, 512], bf16, name="warmzero")
            nc.vector.memset(wz[:], 0.0)
            pw = ps2.tile([128, 512], f32, tag="p2a", name="pwarm")
            for _ in range(3):
                nc.tensor.matmul(
                    pw[:], wz[:, 0:128], wz[:], start=True, stop=True,
                )

            xT_r = xT.rearrange("(k p) b -> p k b", p=128)
            w1t_r = w1t.rearrange("(k p) h -> p k h", p=128)

            KH = K1 // 2

            def load_xt(w):
                """Per-window x tiles: two k-half DMAs + the leftover rows."""
                parts = []
                for ci, (k0, k1e) in enumerate(((0, KH), (KH, K1))):
                    t = xpool.tile([128, k1e - k0, WN], bf16, tag=f"xh{ci}",
                                   name=f"xt_{w}_{ci}")
                    nc.sync.dma_start(t[:], xT_r[:, k0:k1e, w * WN:(w + 1) * WN])
                    parts.append((t, k0, k1e))
                tl = xpool.tile([128, WN], bf16, tag="xl", name=f"xl_{w}")
                if w > 0:
                    nc.sync.dma_start(tl[:], xL[:, w * WN:(w + 1) * WN])
                return parts, tl

            # DMA issue order tracks consumption order: bias pack (tiny,
            # primes the queues), then window-0 x and block-0 weights in
            # fine-grained interleaved 2-chunk pieces so the first L1
            # matmul's dependencies land as early as possible, leftovers,
            # then the remaining W1 blocks interleaved with wbig/w3t.
            bc_sb = cpool.tile([128, 2 * NM], f32)
            nc.sync.dma_start(bc_sb[:], bcat)
            b1_sb = bc_sb[:, 0:NM]
            bb_sb = bc_sb[:, NM:2 * NM]

            w1t_t = [None] * NJ

            def load_w1_block(j):
                parts = []
                for ci, (k0, k1e) in enumerate(((0, KH), (KH, K1))):
                    t = cpool.tile([128, k1e - k0, MB * 128], bf16,
                                   name=f"w1t_{j}_{ci}")
                    nc.sync.dma_start(
                        t[:], w1t_r[:, k0:k1e, j * MB * 128:(j + 1) * MB * 128]
                    )
                    parts.append((t, k0, k1e))
                w1t_t[j] = parts

            # window-0 x and W1 block 0, interleaved k-pieces
            x0_parts = []
            w1b0_parts = []
            for k0, k1e in ((0, 2), (2, 4), (4, K1)):
                tx = cpool.tile([128, k1e - k0, WN], bf16, name=f"x0_{k0}")
                nc.sync.dma_start(tx[:], xT_r[:, k0:k1e, 0:WN])
                x0_parts.append((tx, k0, k1e))
                tw = cpool.tile([128, k1e - k0, MB * 128], bf16, name=f"w1b0_{k0}")
                nc.sync.dma_start(tw[:], w1t_r[:, k0:k1e, 0:MB * 128])
                w1b0_parts.append((tw, k0, k1e))
            w1t_t[0] = w1b0_parts
            xl0 = xpool.tile([128, WN], bf16, tag="xl", name="xl_0")
            xts = {0: (x0_parts, xl0)}

            # window-0 leftover x rows + compact leftover weights (the 16
            # leftover feature rows, replicated into the 4 PE row groups by
            # 4 small DMAs instead of shipping a 128-row mostly-zero tensor)
            nc.sync.dma_start(xl0[:], xL[:, 0:WN])
            w1l_sb = cpool.tile([128, HIDDEN], bf16)
            for j in range(4):
                nc.sync.dma_start(w1l_sb[32 * j:32 * j + KL, :], w1l)

            wbig_sb = cpool.tile([128, NM * 32], bf16)
            w3t_sb = cpool.tile([128, NM * OUT_DIM], bf16)
            load_w1_block(1)
            nc.sync.dma_start(wbig_sb[:], wbig)
            load_w1_block(2)
            nc.sync.dma_start(w3t_sb[:], w3t)
            for j in range(3, NJ):
                load_w1_block(j)

            # Software pipeline, per group G of 4 m-tiles:
            #   L1(G, tiles 0-1) | pair+lft | L1(G, tiles 2-3) | pair+lft | L3
            # Each pair slot is 16 concurrent tile_position matmuls: 8 L2
            # blocks for two of G-1's m-tiles plus the 8 leftover pieces
            # that close the accumulation of two of G's m-tiles.
            NGW = NM // 4
            NGTOT = nw * NGW
            pos = {}   # window -> psum accumulator for layer 3
            hs = {}    # global M -> h tile (layer-1 output)
            h2s = {}   # global M -> h2 tile (layer-2 output)

            def _pick(parts, k):
                for t, k0, k1e in parts:
                    if k0 <= k < k1e:
                        return t, k - k0
                raise KeyError(k)

            def emit_l1_tiles(G, js, p1s):
                """Full-K matmuls for m-tiles {4g+j : j in js} of group G."""
                w, g = divmod(G, NGW)
                xparts, _ = xts[w]
                for j in js:
                    m = 4 * g + j
                    p1 = ps1.tile([128, WN], f32, tag="p1", name=f"p1_{G}_{j}")
                    for k in range(K1):
                        wt, wk = _pick(w1t_t[m // MB], k)
                        xt_c, xk = _pick(xparts, k)
                        nc.tensor.matmul(
                            p1[:],
                            wt[:, wk, (m % MB) * 128:(m % MB + 1) * 128],
                            xt_c[:, xk, :],
                            start=(k == 0),
                            stop=False,
                        )
                    p1s[j] = p1

            def emit_lft(G, js, p1s):
                """Leftover K=16 pieces closing tiles {4g+j : j in js} of G,
                as [16,32] col-strips at the 8 (row,col) grid cells the L2
                pair slot does not occupy (rows c+1 / c+3 for strip c)."""
                w, g = divmod(G, NGW)
                _, xl = xts[w]
                for i, j in enumerate(js):
                    m = 4 * g + j
                    for c in range(4):
                        r = (c + (1 if i == 0 else 3)) % 4
                        nc.tensor.matmul(
                            p1s[j][32 * c:32 * c + 32, :],
                            w1l_sb[32 * r:32 * r + KL,
                                   m * 128 + 32 * c:m * 128 + 32 * c + 32],
                            xl[32 * r:32 * r + KL, :],
                            start=False,
                            stop=True,
                            tile_position=(32 * r, 32 * c),
                            skip_group_check=True,
                        )

            def emit_act(G, js, p1s):
                w, g = divmod(G, NGW)
                for j in js:
                    m = 4 * g + j
                    h = hpool.tile([128, WN], bf16, tag="h", name=f"h_{G}_{j}")
                    nc.scalar.activation(h[:], p1s[j][:], Relu, bias=b1_sb[:, m:m + 1])
                    hs[w * NM + m] = h

            def emit_l2_pair(P):
                """8 concurrent 32x32 block matmuls for m-tiles (2p, 2p+1);
                even tile on the diagonal into bank a, odd tile col-rotated
                by 2 into bank b."""
                w, p = divmod(P, NM // 2)
                m0, m1 = 2 * p, 2 * p + 1
                p2a = ps2.tile([128, WN], f32, tag="p2a", name=f"p2a_{P}")
                p2b = ps2.tile([128, WN], f32, tag="p2b", name=f"p2b_{P}")
                h0 = hs.pop(w * NM + m0)
                h1 = hs.pop(w * NM + m1)
                for j in range(4):
                    nc.tensor.matmul(
                        p2a[32 * j:32 * j + 32, :],
                        wbig_sb[32 * j:32 * j + 32, 32 * m0:32 * m0 + 32],
                        h0[32 * j:32 * j + 32, :],
                        start=True,
                        stop=True,
                        tile_position=(32 * j, 32 * j),
                    )
                for j in range(4):
                    c = (j + 2) % 4
                    nc.tensor.matmul(
                        p2b[32 * c:32 * c + 32, :],
                        wbig_sb[32 * j:32 * j + 32, 32 * m1:32 * m1 + 32],
                        h1[32 * j:32 * j + 32, :],
                        start=True,
                        stop=True,
                        tile_position=(32 * j, 32 * c),
                    )
                for m, p2 in ((m0, p2a), (m1, p2b)):
                    h2 = h2pool.tile([128, WN], bf16, tag="h2", name=f"h2_{P}_{m}")
                    nc.vector.tensor_scalar(h2[:], p2[:], bb_sb[:, m:m + 1], 0.0, Add, Max)
                    h2s[w * NM + m] = h2

            def emit_l3_slot(S):
                """One slot: 4 concurrent M=10 col-tiles; strip t accumulates
                m-tiles == t (mod 4) over the window's 8 slots."""
                w, s = divmod(S, NGW)
                if s == 0:
                    pos[w] = ps3.tile([128, WN], f32, tag="po", name=f"po_{w}")
                for t in range(4):
                    m = 4 * s + t
                    nc.tensor.matmul(
                        pos[w][32 * t:32 * t + OUT_DIM, :],
                        w3t_sb[:, OUT_DIM * m:OUT_DIM * (m + 1)],
                        h2s.pop(w * NM + m)[:],
                        start=(s == 0),
                        stop=(s == NGW - 1),
                        tile_position=(0, 32 * t),
                        skip_group_check=True,
                    )
                if s == NGW - 1:
                    ot = opool.tile([128, WN], f32, tag="ot", name=f"ot_{w}")
                    nc.vector.tensor_scalar_add(ot[:], pos.pop(w)[:], 0.0)
                    nc.sync.dma_start(outT[:, w * WN:(w + 1) * WN], ot[:])

            pending_l3 = []
            for G in range(NGTOT + 2):
                p1s = {}
                if G < NGTOT:
                    w, g = divmod(G, NGW)
                    if g == 0 and w not in xts:
                        xts[w] = load_xt(w)
                    emit_l1_tiles(G, (0, 1), p1s)
                if 1 <= G <= NGTOT:
                    emit_l2_pair(2 * (G - 1))
                if G < NGTOT:
                    emit_lft(G, (0, 1), p1s)
                    emit_act(G, (0, 1), p1s)
                    emit_l1_tiles(G, (2, 3), p1s)
                if 1 <= G <= NGTOT:
                    emit_l2_pair(2 * (G - 1) + 1)
                if G < NGTOT:
                    emit_lft(G, (2, 3), p1s)
                    emit_act(G, (2, 3), p1s)
                if G >= 2:
                    pending_l3.append(G - 2)
                    if len(pending_l3) == 2 or G == NGTOT + 1:
                        for S in pending_l3:
                            emit_l3_slot(S)
                        pending_l3.clear()

    nc.compile()
    return nc


def _get_program(bc=BC):
    if bc not in _PROGRAM_CACHE:
        _PROGRAM_CACHE[bc] = _build_program(bc)
    return _PROGRAM_CACHE[bc]


def _prep_weights(W1, b1, Wb, bb, W3, b3):
    """Host-side packing of replicated weights into device layouts."""
    W1 = np.asarray(W1, dtype=np.float32)
    Wb = np.asarray(Wb, dtype=np.float32)
    W3 = np.asarray(W3, dtype=np.float32)

    # W1T [768, 4096] = first 768 input features; W1L [16, 4096] = the
    # leftover feature rows (replicated into 4 PE row groups device-side).
    W1T = np.ascontiguousarray(W1.T[:K1 * 128]).astype(BF16)
    W1L = np.ascontiguousarray(W1.T[K1 * 128:IN_DIM]).astype(BF16)

    # Wbig [128, 32*32]: partition strip j, col group m holds Wb[4m+j].T.
    Wbig = np.zeros((128, NM * 32), dtype=BF16)
    for m in range(NM):
        for j in range(4):
            Wbig[32 * j:32 * j + 32, 32 * m:32 * m + 32] = Wb[4 * m + j].T.astype(BF16)

    # W3T [128, NM*10]: per m-tile a [128, 10] lhsT with the m-tile's hidden
    # rows in on-chip (possibly rotated) partition order.
    W3T = np.zeros((128, NM * OUT_DIM), dtype=BF16)
    bb = np.asarray(bb, np.float32)
    bb2 = np.zeros((128, NM), dtype=np.float32)
    for m in range(NM):
        pm = 128 * m + _perm(m)
        W3T[:, OUT_DIM * m:OUT_DIM * (m + 1)] = W3[:, pm].T.astype(BF16)
        bb2[:, m] = bb[pm]

    bcat = np.zeros((128, 2 * NM), dtype=np.float32)
    bcat[:, 0:NM] = np.asarray(b1, np.float32).reshape(NM, 128).T
    bcat[:, NM:2 * NM] = bb2
    return dict(W1T=W1T, W1L=W1L, Wbig=Wbig, W3T=W3T, bcat=bcat)


def _prep_x_shard(x, c, ncores=NCORES, bc=BC):
    xs = np.asarray(x[c * bc:(c + 1) * bc], dtype=np.float32).T.astype(BF16)  # [784, bc]
    xT = np.ascontiguousarray(xs[:K1 * 128])
    xLs = np.zeros((128, bc), dtype=BF16)
    for j in range(4):
        xLs[32 * j:32 * j + KL] = xs[K1 * 128:IN_DIM]
    return xT, xLs


def run(x, W1, b1, Wb, bb, W3, b3, trace=False, tmpdir=None):
    """Run on 8 cores; returns (out [B,10] fp32, BassKernelResults)."""
    from concourse.bass_utils import run_bass_kernel_spmd

    nc = _get_program()
    wmap = _prep_weights(W1, b1, Wb, bb, W3, b3)
    in_maps = []
    for c in range(NCORES):
        m = dict(wmap)
        m["xT"], m["xL"] = _prep_x_shard(np.asarray(x), c)
        in_maps.append(m)

    res = run_bass_kernel_spmd(
        nc, in_maps, core_ids=list(range(NCORES)), trace=trace, tmpdir=tmpdir
    )
    b3f = np.asarray(b3, np.float32)
    outs = []
    for r in res.results:
        raw = np.asarray(r["outT"])  # [128, BC]; strips at rows 32t..32t+10
        acc = sum(raw[32 * t:32 * t + OUT_DIM] for t in range(4))
        outs.append(acc.T + b3f)
    out = np.concatenate(outs, axis=0).astype(np.float32)
    return out, res


def kernel(x, W1, b1, Wb, bb, W3, b3):
    out, _ = run(x, W1, b1, Wb, bb, W3, b3, trace=False)
    return out


# revision 16
# speedup vs baseline: 1.0629x; 1.0033x over previous
"""Trainium2 Bass kernel for BlockDiagMNIST MLP.

Reference computation (all fp32):
    h  = relu(x @ W1.T + b1)          x:[B,784], W1:[4096,784]    -> [B,4096]
    yb = blockdiag(h, Wb)             Wb:[128,32,32] (h2[b, 32n+o] = sum_k h[b,32n+k] Wb[n,o,k])
    h2 = relu(yb + bb)
    out = h2 @ W3.T + b3              W3:[10,4096]                -> [B,10]

Strategy: pure data-parallel over batch (B=32768 -> 4096 rows/core on 8 cores),
weights replicated.  All matmuls in bf16 (fp32 PSUM accumulation, fp32 biases).
On-chip layout is transposed ("hidden on partitions"): we compute
hT = W1 @ x.T per 512-column batch window.

PE-array packing: layers 2 and 3 underutilize the 128x128 array (32x32
diagonal blocks / M=10 output), so they are packed as concurrent
tile_position matmuls sharing one streaming slot:
  - L2: two m-tiles (8 32x32 blocks) per slot on a (row,col)-disjoint grid;
    the odd tile's outputs land column-rotated in a second PSUM bank, and the
    rotation is absorbed into the host-side packing of bb and W3.
  - The leftover K=16 matmuls ride in the same pair slots as [16,32]
    col-strip pieces at the 8 unused (row,col) positions, closing each
    L1 accumulation without a dedicated slot.
  - L3: four K=128 chunks per slot at col positions (0,32t); each 10-row
    strip accumulates a quarter of the hidden sum, and the four partial
    strips are summed (+b3) on the host.

Host-side prep (free - not on the device timeline): transpose + bf16-cast of
x and weights, packing Wb into 32x32 lhsT strips.
"""

import numpy as np
import ml_dtypes

B = 32768
IN_DIM = 784
HIDDEN = 4096
BLOCK = 32
NUM_BLOCKS = 128
OUT_DIM = 10
NCORES = 8
BC = B // NCORES          # batch rows per core (4096)
WN = 512                  # batch-window columns (one matmul free-dim)
K1 = 6                    # full 128-row K-chunks for layer 1 (features 0..767)
KL = 16                   # leftover K rows (features 768..783), row-group packed
NM = HIDDEN // 128        # 32 hidden tiles (also layer-3 K-chunks)

BF16 = ml_dtypes.bfloat16

_PROGRAM_CACHE = {}


def _perm(m):
    """Partition permutation of hidden rows within m-tile m as laid out on chip.

    Even m-tiles are natural; odd m-tiles are emitted by L2 with their four
    32-row blocks rotated two col-groups to keep the 8 concurrent tile
    positions disjoint: psum partition p = 32c+i holds hidden row
    128m + 32((c+2)%4) + i.
    """
    p = np.arange(128)
    if m % 2 == 0:
        return p
    return 32 * ((p // 32 + 2) % 4) + p % 32


def _build_program(bc=BC):
    """Build (and bacc-compile) the per-core Bass program. bc = batch cols/core."""
    import concourse.mybir as mybir
    import concourse.tile as tile
    from concourse import bacc

    nw = bc // WN
    f32, bf16 = mybir.dt.float32, mybir.dt.bfloat16

    nc = bacc.Bacc("TRN2", target_bir_lowering=False, debug=False)

    xT = nc.dram_tensor("xT", [K1 * 128, bc], bf16, kind="ExternalInput").ap()
    xL = nc.dram_tensor("xL", [128, bc], bf16, kind="ExternalInput").ap()
    w1t = nc.dram_tensor("W1T", [K1 * 128, HIDDEN], bf16, kind="ExternalInput").ap()
    w1l = nc.dram_tensor("W1L", [KL, HIDDEN], bf16, kind="ExternalInput").ap()
    wbig = nc.dram_tensor("Wbig", [128, NM * 32], bf16, kind="ExternalInput").ap()
    w3t = nc.dram_tensor("W3T", [128, NM * OUT_DIM], bf16, kind="ExternalInput").ap()
    # biases packed into one tensor: cols 0..NM-1 = b1, NM..2NM-1 = bb (permuted)
    bcat = nc.dram_tensor("bcat", [128, 2 * NM], f32, kind="ExternalInput").ap()
    # raw PSUM image per window; rows 32t..32t+10 hold partial strip t
    # (summed host-side), other rows are garbage.
    outT = nc.dram_tensor("outT", [128, bc], f32, kind="ExternalOutput").ap()

    Relu = mybir.ActivationFunctionType.Relu
    Add = mybir.AluOpType.add
    Max = mybir.AluOpType.max

    MB = 4          # W1T column-block = MB m-tiles (DMA granularity for overlap)
    NJ = NM // MB   # 8 column blocks

    with tile.TileContext(nc) as tc:
        with (
            tc.tile_pool(name="const", bufs=1) as cpool,
            tc.tile_pool(name="xin", bufs=3) as xpool,
            tc.tile_pool(name="hbuf", bufs=10) as hpool,
            tc.tile_pool(name="h2buf", bufs=14) as h2pool,
            tc.tile_pool(name="obuf", bufs=2) as opool,
            tc.tile_pool(name="ps1", bufs=5, space="PSUM") as ps1,
            tc.tile_pool(name="ps2", bufs=1, space="PSUM") as ps2,
            tc.tile_pool(name="ps3", bufs=1, space="PSUM") as ps3,
        ):
            # HAM warmup on a memset tile: no DMA dependency, so the PE
            # starts as soon as its program is loaded (~8us); a few bf16
            # N=512 matmuls bridge until the first x/W1 DMA pieces land.
            wz = cpool.tile([128, 512], bf16, name="warmzero")
            nc.vector.memset(wz[:], 0.0)
            pw = ps2.tile([128, 512], f32, tag="p2a", name="pwarm")
            for _ in range(3):
                nc.tensor.matmul(
                    pw[:], wz[:, 0:128], wz[:], start=True, stop=True,
                )

            xT_r = xT.rearrange("(k p) b -> p k b", p=128)
            w1t_r = w1t.rearrange("(k p) h -> p k h", p=128)

            KH = K1 // 2

            def load_xt(w):
                """Per-window x tiles: two k-half DMAs + the leftover rows."""
                parts = []
                for ci, (k0, k1e) in enumerate(((0, KH), (KH, K1))):
                    t = xpool.tile([128, k1e - k0, WN], bf16, tag=f"xh{ci}",
                                   name=f"xt_{w}_{ci}")
                    nc.sync.dma_start(t[:], xT_r[:, k0:k1e, w * WN:(w + 1) * WN])
                    parts.append((t, k0, k1e))
                tl = xpool.tile([128, WN], bf16, tag="xl", name=f"xl_{w}")
                if w > 0:
                    nc.sync.dma_start(tl[:], xL[:, w * WN:(w + 1) * WN])
                return parts, tl

            # DMA issue order tracks consumption order: bias pack (tiny,
            # primes the queues), then window-0 x and block-0 weights in
            # fine-grained interleaved 2-chunk pieces so the first L1
            # matmul's dependencies land as early as possible, leftovers,
            # then the remaining W1 blocks interleaved with wbig/w3t.
            bc_sb = cpool.tile([128, 2 * NM], f32)
            nc.sync.dma_start(bc_sb[:], bcat)
            b1_sb = bc_sb[:, 0:NM]
            bb_sb = bc_sb[:, NM:2 * NM]

            w1t_t = [None] * NJ

            def load_w1_block(j):
                parts = []
                for ci, (k0, k1e) in enumerate(((0, KH), (KH, K1))):
                    t = cpool.tile([128, k1e - k0, MB * 128], bf16,
                                   name=f"w1t_{j}_{ci}")
                    nc.sync.dma_start(
                        t[:], w1t_r[:, k0:k1e, j * MB * 128:(j + 1) * MB * 128]
                    )
                    parts.append((t, k0, k1e))
                w1t_t[j] = parts

            # window-0 x and W1 block 0, interleaved k-pieces
            x0_parts = []
            w1b0_parts = []
            for k0, k1e in ((0, 2), (2, 4), (4, K1)):
                tx = cpool.tile([128, k1e - k0, WN], bf16, name=f"x0_{k0}")
                nc.sync.dma_start(tx[:], xT_r[:, k0:k1e, 0:WN])
                x0_parts.append((tx, k0, k1e))
                tw = cpool.tile([128, k1e - k0, MB * 128], bf16, name=f"w1b0_{k0}")
                nc.sync.dma_start(tw[:], w1t_r[:, k0:k1e, 0:MB * 128])
                w1b0_parts.append((tw, k0, k1e))
            w1t_t[0] = w1b0_parts
            xl0 = xpool.tile([128, WN], bf16, tag="xl", name="xl_0")
            xts = {0: (x0_parts, xl0)}

            # window-0 leftover x rows + compact leftover weights (the 16
            # leftover feature rows, replicated into the 4 PE row groups by
            # 4 small DMAs instead of shipping a 128-row mostly-zero tensor)
            nc.sync.dma_start(xl0[:], xL[:, 0:WN])
            w1l_sb = cpool.tile([128, HIDDEN], bf16)
            for j in range(4):
                nc.sync.dma_start(w1l_sb[32 * j:32 * j + KL, :], w1l)

            wbig_sb = cpool.tile([128, NM * 32], bf16)
            w3t_sb = cpool.tile([128, NM * OUT_DIM], bf16)
            load_w1_block(1)
            nc.sync.dma_start(wbig_sb[:], wbig)
            load_w1_block(2)
            nc.sync.dma_start(w3t_sb[:], w3t)
            for j in range(3, NJ):
                load_w1_block(j)

            # Software pipeline, per group G of 4 m-tiles:
            #   L1(G, tiles 0-1) | pair+lft | L1(G, tiles 2-3) | pair+lft | L3
            # Each pair slot is 16 concurrent tile_position matmuls: 8 L2
            # blocks for two of G-1's m-tiles plus the 8 leftover pieces
            # that close the accumulation of two of G's m-tiles.
            NGW = NM // 4
            NGTOT = nw * NGW
            pos = {}   # window -> psum accumulator for layer 3
            hs = {}    # global M -> h tile (layer-1 output)
            h2s = {}   # global M -> h2 tile (layer-2 output)

            def _pick(parts, k):
                for t, k0, k1e in parts:
                    if k0 <= k < k1e:
                        return t, k - k0
                raise KeyError(k)

            def emit_l1_tiles(G, js, p1s):
                """Full-K matmuls for m-tiles {4g+j : j in js} of group G."""
                w, g = divmod(G, NGW)
                xparts, _ = xts[w]
                for j in js:
                    m = 4 * g + j
                    p1 = ps1.tile([128, WN], f32, tag="p1", name=f"p1_{G}_{j}")
                    for k in range(K1):
                        wt, wk = _pick(w1t_t[m // MB], k)
                        xt_c, xk = _pick(xparts, k)
                        nc.tensor.matmul(
                            p1[:],
                            wt[:, wk, (m % MB) * 128:(m % MB + 1) * 128],
                            xt_c[:, xk, :],
                            start=(k == 0),
                            stop=False,
                        )
                    p1s[j] = p1

            def emit_lft(G, js, p1s):
                """Leftover K=16 pieces closing tiles {4g+j : j in js} of G,
                as [16,32] col-strips at the 8 (row,col) grid cells the L2
                pair slot does not occupy (rows c+1 / c+3 for strip c)."""
                w, g = divmod(G, NGW)
                _, xl = xts[w]
                for i, j in enumerate(js):
                    m = 4 * g + j
                    for c in range(4):
                        r = (c + (1 if i == 0 else 3)) % 4
                        nc.tensor.matmul(
                            p1s[j][32 * c:32 * c + 32, :],
                            w1l_sb[32 * r:32 * r + KL,
                                   m * 128 + 32 * c:m * 128 + 32 * c + 32],
                            xl[32 * r:32 * r + KL, :],
                            start=False,
                            stop=True,
                            tile_position=(32 * r, 32 * c),
                            skip_group_check=True,
                        )

            def emit_act(G, js, p1s):
                w, g = divmod(G, NGW)
                for j in js:
                    m = 4 * g + j
                    h = hpool.tile([128, WN], bf16, tag="h", name=f"h_{G}_{j}")
                    nc.scalar.activation(h[:], p1s[j][:], Relu, bias=b1_sb[:, m:m + 1])
                    hs[w * NM + m] = h

            def emit_l2_pair(P):
                """8 concurrent 32x32 block matmuls for m-tiles (2p, 2p+1);
                even tile on the diagonal into bank a, odd tile col-rotated
                by 2 into bank b."""
                w, p = divmod(P, NM // 2)
                m0, m1 = 2 * p, 2 * p + 1
                p2a = ps2.tile([128, WN], f32, tag="p2a", name=f"p2a_{P}")
                p2b = ps2.tile([128, WN], f32, tag="p2b", name=f"p2b_{P}")
                h0 = hs.pop(w * NM + m0)
                h1 = hs.pop(w * NM + m1)
                for j in range(4):
                    nc.tensor.matmul(
                        p2a[32 * j:32 * j + 32, :],
                        wbig_sb[32 * j:32 * j + 32, 32 * m0:32 * m0 + 32],
                        h0[32 * j:32 * j + 32, :],
                        start=True,
                        stop=True,
                        tile_position=(32 * j, 32 * j),
                    )
                for j in range(4):
                    c = (j + 2) % 4
                    nc.tensor.matmul(
                        p2b[32 * c:32 * c + 32, :],
                        wbig_sb[32 * j:32 * j + 32, 32 * m1:32 * m1 + 32],
                        h1[32 * j:32 * j + 32, :],
                        start=True,
                        stop=True,
                        tile_position=(32 * j, 32 * c),
                    )
                for m, p2 in ((m0, p2a), (m1, p2b)):
                    h2 = h2pool.tile([128, WN], bf16, tag="h2", name=f"h2_{P}_{m}")
                    nc.vector.tensor_scalar(h2[:], p2[:], bb_sb[:, m:m + 1], 0.0, Add, Max)
                    h2s[w * NM + m] = h2

            def emit_l3_slot(S):
                """One slot: 4 concurrent M=10 col-tiles; strip t accumulates
                m-tiles == t (mod 4) over the window's 8 slots."""
                w, s = divmod(S, NGW)
                if s == 0:
                    pos[w] = ps3.tile([128, WN], f32, tag="po", name=f"po_{w}")
                for t in range(4):
                    m = 4 * s + t
                    nc.tensor.matmul(
                        pos[w][32 * t:32 * t + OUT_DIM, :],
                        w3t_sb[:, OUT_DIM * m:OUT_DIM * (m + 1)],
                        h2s.pop(w * NM + m)[:],
                        start=(s == 0),
                        stop=(s == NGW - 1),
                        tile_position=(0, 32 * t),
                        skip_group_check=True,
                    )
                if s == NGW - 1:
                    ot = opool.tile([128, WN], f32, tag="ot", name=f"ot_{w}")
                    nc.vector.tensor_scalar_add(ot[:], pos.pop(w)[:], 0.0)
                    nc.sync.dma_start(outT[:, w * WN:(w + 1) * WN], ot[:])

            pending_l3 = []
            for G in range(NGTOT + 2):
                p1s = {}
                if G < NGTOT:
                    w, g = divmod(G, NGW)
                    if g == 0 and w not in xts:
                        xts[w] = load_xt(w)
                    emit_l1_tiles(G, (0, 1), p1s)
                if 1 <= G <= NGTOT:
                    emit_l2_pair(2 * (G - 1))
                if G < NGTOT:
                    emit_lft(G, (0, 1), p1s)
                    emit_act(G, (0, 1), p1s)
                    emit_l1_tiles(G, (2, 3), p1s)
                if 1 <= G <= NGTOT:
                    emit_l2_pair(2 * (G - 1) + 1)
                if G < NGTOT:
                    emit_lft(G, (2, 3), p1s)
                    emit_act(G, (2, 3), p1s)
                if G >= 2:
                    pending_l3.append(G - 2)
                    if len(pending_l3) == 2 or G == NGTOT + 1:
                        for S in pending_l3:
                            emit_l3_slot(S)
                        pending_l3.clear()

    nc.compile()
    return nc


def _get_program(bc=BC):
    if bc not in _PROGRAM_CACHE:
        _PROGRAM_CACHE[bc] = _build_program(bc)
    return _PROGRAM_CACHE[bc]


def _prep_weights(W1, b1, Wb, bb, W3, b3):
    """Host-side packing of replicated weights into device layouts."""
    W1 = np.asarray(W1, dtype=np.float32)
    Wb = np.asarray(Wb, dtype=np.float32)
    W3 = np.asarray(W3, dtype=np.float32)

    # W1T [768, 4096] = first 768 input features; W1L [16, 4096] = the
    # leftover feature rows (replicated into 4 PE row groups device-side).
    W1T = np.ascontiguousarray(W1.T[:K1 * 128]).astype(BF16)
    W1L = np.ascontiguousarray(W1.T[K1 * 128:IN_DIM]).astype(BF16)

    # Wbig [128, 32*32]: partition strip j, col group m holds Wb[4m+j].T.
    Wbig = np.zeros((128, NM * 32), dtype=BF16)
    for m in range(NM):
        for j in range(4):
            Wbig[32 * j:32 * j + 32, 32 * m:32 * m + 32] = Wb[4 * m + j].T.astype(BF16)

    # W3T [128, NM*10]: per m-tile a [128, 10] lhsT with the m-tile's hidden
    # rows in on-chip (possibly rotated) partition order.
    W3T = np.zeros((128, NM * OUT_DIM), dtype=BF16)
    bb = np.asarray(bb, np.float32)
    bb2 = np.zeros((128, NM), dtype=np.float32)
    for m in range(NM):
        pm = 128 * m + _perm(m)
        W3T[:, OUT_DIM * m:OUT_DIM * (m + 1)] = W3[:, pm].T.astype(BF16)
        bb2[:, m] = bb[pm]

    bcat = np.zeros((128, 2 * NM), dtype=np.float32)
    bcat[:, 0:NM] = np.asarray(b1, np.float32).reshape(NM, 128).T
    bcat[:, NM:2 * NM] = bb2
    return dict(W1T=W1T, W1L=W1L, Wbig=Wbig, W3T=W3T, bcat=bcat)


def _prep_x_shard(x, c, ncores=NCORES, bc=BC):
    xs = np.asarray(x[c * bc:(c + 1) * bc], dtype=np.float32).T.astype(BF16)  # [784, bc]
    xT = np.ascontiguousarray(xs[:K1 * 128])
    xLs = np.zeros((128, bc), dtype=BF16)
    for j in range(4):
        xLs[32 * j:32 * j + KL] = xs[K1 * 128:IN_DIM]
    return xT, xLs


def run(x, W1, b1, Wb, bb, W3, b3, trace=False, tmpdir=None):
    """Run on 8 cores; returns (out [B,10] fp32, BassKernelResults)."""
    from concourse.bass_utils import run_bass_kernel_spmd

    nc = _get_program()
    wmap = _prep_weights(W1, b1, Wb, bb, W3, b3)
    in_maps = []
    for c in range(NCORES):
        m = dict(wmap)
        m["xT"], m["xL"] = _prep_x_shard(np.asarray(x), c)
        in_maps.append(m)

    res = run_bass_kernel_spmd(
        nc, in_maps, core_ids=list(range(NCORES)), trace=trace, tmpdir=tmpdir
    )
    b3f = np.asarray(b3, np.float32)
    outs = []
    for r in res.results:
        raw = np.asarray(r["outT"])  # [128, BC]; strips at rows 32t..32t+10
        acc = sum(raw[32 * t:32 * t + OUT_DIM] for t in range(4))
        outs.append(acc.T + b3f)
    out = np.concatenate(outs, axis=0).astype(np.float32)
    return out, res


def kernel(x, W1, b1, Wb, bb, W3, b3):
    out, _ = run(x, W1, b1, Wb, bb, W3, b3, trace=False)
    return out
